# revision 17
# baseline (speedup 1.0000x reference)
"""Trainium2 Bass kernel for nn_Abcnn2Portion (ABCNN-2 attention pooling).

Shapes (hardcoded): B=16, N=259 (L=256 + W-1=3), H=128, W=4, EPS=1e-6.
Reference:
    att[b,i,j] = 1 / (1 + sqrt(||x1[b,0,j,:] - x2[b,0,i,:]||^2 + EPS))
    x1_a[b,j] = sum_i att[b,i,j];  x2_a[b,i] = sum_j att[b,i,j]
    out_t[b,0,l,:] = sum_{k=0..3} x_t[b,0,l+k,:] * a_t[b,l+k],  l in [0,256)
Returns (out1, out2), each (16,1,256,128) fp32.

Strategy: data-parallel over batch, 2 batches per core across 8 cores.
Per batch: squared distances via Gram matmul on the PE
(s2 = sq1[j] + sq2[i] - 2*x2.x1, all three terms accumulated in PSUM by
three matmuls), sqrt on the scalar engine, fused (1/(1+x)) + row-sum as a
single custom DVE op, column sums via ones-matmul, and the W=4 sliding
window pooling as a banded-matrix matmul producing the output layout
directly.
"""

import numpy as np

import concourse.bass as bass
import concourse.tile as tile
from concourse import mybir
from concourse.bass_utils import run_bass_kernel_spmd

# --------------------------------------------------------------------------
# Custom DVE op: out = approx(1/(1 + x)) (seed + 1 Newton step, ~0.17% max
# rel err, zero-mean), accum_out = sum(out, free axis).
# --------------------------------------------------------------------------
import concourse.dve_ops as dve_ops
from concourse.dve_spec import Spec, Src0, C0, C1, One, AluOp, Bin, lower, _has_src1
from concourse.dve_ops import DveOp, OPS
from concourse.dve_uop import DveOpSpec

_S = Src0 + One
_nt = Bin(AluOp.BITWISE_NOT, _S, _S)
_y0 = _nt * C0
_BODY = _y0 * (C1 - _S * _y0)


def _recip_ref(in0, in1, s0, s1, imm2):
    S = (in0.astype(np.float32) + np.float32(1.0)).astype(np.float32)
    nt = (~S.view(np.int32)).view(np.float32)
    y0 = nt * np.float32(s0)
    return y0 * (np.float32(s1) - S * y0)


def _register_recip_op():
    name = "ADD1_RECIP_SUM_ANT"
    for existing in OPS:
        if existing.name == name:
            return existing
    spec = Spec(body=_BODY, accum=AluOp.ADD, reference=_recip_ref)
    op = DveOp(name, spec, subdim=False, uops_sha={})
    OPS.append(op)
    dve_ops._SUB_OPCODE_FOR_NAME[name] = dve_ops._CUSTOM_DVE_ROW_BASE + len(OPS) - 1
    for ver in ("v3", "v4"):
        op.uops_sha[ver] = DveOpSpec(
            name=name,
            opcode=dve_ops.get_dve_sub_opcode(name),
            uops=lower(spec, ver=ver),
            rd1_en=_has_src1(spec),
        ).sha(ver)
    return op


RECIP_OP = _register_recip_op()
RECIP_C0 = -0.23549792
RECIP_C1 = 2.0017324

# --------------------------------------------------------------------------
# Problem constants
# --------------------------------------------------------------------------
B, L, W, H = 16, 256, 4, 128
N = L + W - 1  # 259
EPS = 1e-6
NCORES = 8
BPC = B // NCORES  # batches per core = 2

f32 = mybir.dt.float32
f32r = mybir.dt.float32r
AF = mybir.ActivationFunctionType
ALU = mybir.AluOpType

# i-chunk decomposition of N=259: two full 128-partition chunks + 3 leftover
CHUNKS = [(0, 128), (128, 128), (256, 3)]
# f32r matmuls require an even moving (free) dim; pad 259 -> 260. The padded
# column holds garbage and is excluded everywhere it would matter.
NP = 260


def _host_consts():
    """DMA-able constant tables: identity, band main, band boundary."""
    ident = np.eye(128, dtype=np.float32)
    band = np.zeros((128, 128), dtype=np.float32)  # band[n, l] = 1 if l<=n<=l+3
    for n in range(128):
        lo = max(0, n - (W - 1))
        band[n, lo : n + 1] = 1.0
    bandb = np.zeros((128, 128), dtype=np.float32)
    for r in range(W - 1):  # boundary rows n = 128+r relative to l in [0,128)
        bandb[r, 125 + r : 128] = 1.0
    return np.stack([ident, band, bandb])  # (3, 128, 128)


def build_nc():
    nc = bass.Bass()
    x1_in = nc.dram_tensor("x1", [BPC, 1, N, H], f32, kind="ExternalInput")
    x2_in = nc.dram_tensor("x2", [BPC, 1, N, H], f32, kind="ExternalInput")
    consts_in = nc.dram_tensor("consts", [3, 128, 128], f32, kind="ExternalInput")
    out1_d = nc.dram_tensor("out1", [BPC, 1, L, H], f32, kind="ExternalOutput")
    out2_d = nc.dram_tensor("out2", [BPC, 1, L, H], f32, kind="ExternalOutput")

    xin = {0: x1_in, 1: x2_in}
    outd = {0: out1_d, 1: out2_d}

    with tile.TileContext(nc) as tc:
        with (
            tc.tile_pool(name="singles", bufs=1) as singles,
            tc.tile_pool(name="work", bufs=1) as work,
            tc.tile_pool(name="epool", bufs=2) as epool,
            tc.tile_pool(name="attpool", bufs=3) as attpool,
            tc.tile_pool(name="tp_ps", bufs=2, space="PSUM") as tp_ps,
            tc.tile_pool(name="gram_ps", bufs=3, space="PSUM") as gram_ps,
            tc.tile_pool(name="row_ps", bufs=1, space="PSUM") as row_ps,
            tc.tile_pool(name="acol_ps", bufs=1, space="PSUM") as acol_ps,
            tc.tile_pool(name="band_ps", bufs=1, space="PSUM") as band_ps,
        ):
            # ---- constants ----
            ident = singles.tile([128, 128], f32, tag="ident")
            band32 = singles.tile([128, 128], f32, tag="band32")
            bandb32 = singles.tile([3, 128], f32, tag="bandb32")
            band = singles.tile([128, 128], f32r, tag="band")
            bandb = singles.tile([3, 128], f32r, tag="bandb")
            ones = singles.tile([128, 512], f32r, tag="ones")
            ones32 = singles.tile([1, 1], f32, tag="ones32")
            epsb = singles.tile([128, 1], f32, tag="epsb")
            nc.sync.dma_start(out=ident, in_=consts_in[0])
            nc.sync.dma_start(out=band32, in_=consts_in[1])
            nc.sync.dma_start(out=bandb32, in_=consts_in[2, 0:3, :])
            nc.vector.tensor_copy(band[:, :], band32[:, :])
            nc.vector.tensor_copy(bandb[:, :], bandb32[:, :])
            ones_f = singles.tile([128, 512], f32, tag="ones_f")
            nc.gpsimd.memset(ones_f[:, :], 1.0)
            nc.vector.tensor_copy(ones[:, :], ones_f[:, :])
            nc.gpsimd.memset(ones32[:, :], 1.0)
            nc.gpsimd.memset(epsb[:, :], EPS)

            # ---- inputs, natural layout ----
            # x_nd[t]: (p, b, c, h) with n = c*128 + p   (c = 0,1)
            # x_l[t]:  (r, b, h)    with n = 256 + r
            x_nd, x_l = {}, {}
            for t in (0, 1):
                x_nd[t] = work.tile([128, BPC, 2, H], f32, tag=f"x{t}nd", name=f"x{t}nd")
                x_l[t] = work.tile([3, BPC, H], f32, tag=f"x{t}l", name=f"x{t}l")
                for b in range(BPC):
                    nc.sync.dma_start(
                        out=x_nd[t][:, b, :, :],
                        in_=xin[t][b, 0, 0:L, :].rearrange(
                            "(c p) h -> p c h", c=2
                        ),
                    )
                    nc.sync.dma_start(
                        out=x_l[t][:, b, :], in_=xin[t][b, 0, L:N, :]
                    )

            # ---- transposes to d-layout + squares, per (tensor, batch) ----
            # x_dn[t][b]: (128, 259)  d on partitions
            # xsq[t][b]:  (128, 259)  elementwise square of x_dn
            # x2 path is pre-scaled by -2 for the Gram term.
            x_dn = {0: {}, 1: {}}
            xsq = {0: {}, 1: {}}
            for t in (0, 1):
                for b in range(BPC):
                    tp = tp_ps.tile([128, N], f32, tag="tp")
                    for c in (0, 1):
                        nc.tensor.transpose(
                            tp[:, c * 128 : (c + 1) * 128],
                            x_nd[t][:, b, c, :],
                            ident[:, :],
                        )
                    nc.tensor.transpose(
                        tp[:, 256:259], x_l[t][:, b, :], ident[0:3, 0:3]
                    )
                    dn = work.tile([128, NP], f32r, tag=f"x{t}dn{b}")
                    sq = work.tile([128, NP], f32r, tag=f"x{t}sq{b}")
                    if t == 0:
                        nc.scalar.copy(dn[:, 0:N], tp[:, :])
                    else:
                        # -2 * x2 in d-layout (DVE, psum->sbuf)
                        nc.vector.tensor_scalar(
                            out=dn[:, 0:N], in0=tp[:, :],
                            scalar1=-2.0, scalar2=None, op0=ALU.mult,
                        )
                    nc.scalar.activation(sq[:, 0:N], tp[:, :], AF.Square)
                    x_dn[t][b] = dn
                    xsq[t][b] = sq

            # ---- attention chunks ----
            # psum = -2*x2.x1 + sq1[j] + sq2[i]; e = sqrt(psum + EPS)
            # att = 1/(1+e) (custom op) with accum -> x2_a columns
            # colsum ones-matmul -> x1_a row
            a_cols = {0: {}, 1: {}}  # a_cols[t][b]: (128, 4) sbuf, cols=chunks
            for t in (0, 1):
                for b in range(BPC):
                    a_cols[t][b] = work.tile([128, 4], f32, tag=f"a{t}c{b}", name=f"a{t}c{b}")

            x1row_sb = {}
            for b in range(BPC):
                rowp = row_ps.tile([1, NP], f32, tag="x1row")
                for ci, (i0, P) in enumerate(CHUNKS):
                    g = gram_ps.tile([128, NP], f32, tag="gram")
                    # -2 * x2[:,i] . x1[:,j]
                    nc.tensor.matmul(
                        g[0:P, :],
                        x_dn[1][b][:, i0 : i0 + P],
                        x_dn[0][b][:, :],
                        start=True, stop=False,
                    )
                    # + sq1[j] broadcast over i  (ones^T @ xsq1)
                    nc.tensor.matmul(
                        g[0:P, :],
                        ones[:, 0:P],
                        xsq[0][b][:, :],
                        start=False, stop=False,
                    )
                    # + sq2[i] broadcast over j  (xsq2_chunk^T @ ones)
                    nc.tensor.matmul(
                        g[0:P, :],
                        xsq[1][b][:, i0 : i0 + P],
                        ones[:, 0:NP],
                        start=False, stop=True,
                    )
                    e = epool.tile([128, NP], f32, tag="e")
                    nc.scalar.activation(
                        e[0:P, :], g[0:P, :], AF.Sqrt, bias=epsb[0:P, 0:1]
                    )
                    att = attpool.tile([128, NP], f32r, tag="att")
                    nc.vector._custom_dve(
                        RECIP_OP,
                        out=att[0:P, 0:N], in0=e[0:P, 0:N],
                        s0=RECIP_C0, s1=RECIP_C1,
                        accum_out=a_cols[1][b][0:P, ci : ci + 1],
                    )

                    # x1_a row accumulation: ones_col^T @ att
                    nc.tensor.matmul(
                        rowp[:, :],
                        ones[0:P, 0:1],
                        att[0:P, :],
                        start=(ci == 0), stop=(ci == 2),
                    )
                row_sb = work.tile([1, NP], f32, tag="x1row_sb")
                nc.scalar.copy(row_sb[:, :], rowp[:, :])
                x1row_sb[b] = row_sb

            # x1_a row -> per-partition columns via tiny K=1 matmuls
            for b in range(BPC):
                ac = acol_ps.tile([128, 4], f32, tag="acolp")
                for ci, (i0, P) in enumerate(CHUNKS):
                    nc.tensor.matmul(
                        ac[0:P, ci : ci + 1],
                        x1row_sb[b][:, i0 : i0 + P],
                        ones32[0:1, 0:1],
                        start=True, stop=True,
                    )
                nc.vector.tensor_copy(a_cols[0][b][:, :], ac[:, :])

            # ---- weighted sliding-window pooling via banded matmul ----
            # wx = x * a (per-partition scalars); out[l, b, h] accumulated in
            # one psum bank per tensor, then one copy + one DMA out.
            for t in (0, 1):
                wx = work.tile([128, BPC, 2, H], f32r, tag=f"wx{t}", name=f"wx{t}")
                wxl = work.tile([3, BPC, H], f32r, tag=f"wxl{t}", name=f"wxl{t}")
                for b in range(BPC):
                    for c in (0, 1):
                        nc.gpsimd.tensor_scalar(
                            out=wx[:, b, c, :], in0=x_nd[t][:, b, c, :],
                            scalar1=a_cols[t][b][:, c : c + 1],
                            scalar2=None, op0=ALU.mult,
                        )
                    nc.gpsimd.tensor_scalar(
                        out=wxl[:, b, :], in0=x_l[t][:, b, :],
                        scalar1=a_cols[t][b][0:3, 2:3],
                        scalar2=None, op0=ALU.mult,
                    )
                bp = band_ps.tile([128, 2, BPC, H], f32, tag="bandp")
                # l-chunk 0: main rows n=0..127 (wx c=0), boundary n=128..130
                nc.tensor.matmul(
                    bp[:, 0, :, :],
                    band[:, :],
                    wx[:, :, 0, :],
                    start=True, stop=False,
                )
                nc.tensor.matmul(
                    bp[:, 0, :, :],
                    bandb[0:3, :],
                    wx[0:3, :, 1, :],
                    start=False, stop=True,
                )
                # l-chunk 1: main rows n=128..255 (wx c=1), boundary n=256..258
                nc.tensor.matmul(
                    bp[:, 1, :, :],
                    band[:, :],
                    wx[:, :, 1, :],
                    start=True, stop=False,
                )
                nc.tensor.matmul(
                    bp[:, 1, :, :],
                    bandb[0:3, :],
                    wxl[:, :, :],
                    start=False, stop=True,
                )
                osb = work.tile([128, 2, BPC, H], f32, tag=f"osb{t}")
                nc.vector.tensor_copy(osb[:, :, :, :], bp[:, :, :, :])
                for b in range(BPC):
                    nc.sync.dma_start(
                        out=outd[t][b, 0, :, :].rearrange(
                            "(lc p) h -> p lc h", lc=2
                        ),
                        in_=osb[:, :, b, :],
                    )
    # TRN2 allows at most 1 sem wait per instruction (2 on EventSemaphore);
    # Tile can attach more — split them like Bacc.compile does.
    import bass_rust
    from concourse import mybir as _mybir
    bass_rust.generate_event_semaphores(nc)
    _mybir.codegen_inst_isa_subclasses(nc)
    return nc


_NC_CACHE = {}


def _get_nc():
    if "nc" not in _NC_CACHE:
        _NC_CACHE["nc"] = build_nc()
    return _NC_CACHE["nc"]


def _run(x1, x2, **kwargs):
    x1 = np.ascontiguousarray(np.asarray(x1), dtype=np.float32)
    x2 = np.ascontiguousarray(np.asarray(x2), dtype=np.float32)
    consts = _host_consts()
    nc = _get_nc()
    core_ids = list(range(NCORES))
    in_maps = [
        {
            "x1": x1[c * BPC : (c + 1) * BPC],
            "x2": x2[c * BPC : (c + 1) * BPC],
            "consts": consts,
        }
        for c in core_ids
    ]
    br = run_bass_kernel_spmd(nc, in_maps, core_ids, **kwargs)
    out1 = np.concatenate([r["out1"] for r in br.results], axis=0)
    out2 = np.concatenate([r["out2"] for r in br.results], axis=0)
    return (out1, out2), br


def kernel(x1, x2):
    (out1, out2), _ = _run(x1, x2)
    return (out1, out2)


if __name__ == "__main__":
    rng = np.random.default_rng(0)
    x1 = rng.standard_normal((B, 1, N, H)).astype(np.float32)
    x2 = rng.standard_normal((B, 1, N, H)).astype(np.float32)
    o1, o2 = kernel(x1, x2)
    print("out shapes:", o1.shape, o2.shape)


# revision 19
# speedup vs baseline: 1.3971x; 1.3971x over previous
"""Trainium2 Bass kernel for nn_Abcnn2Portion (ABCNN-2 attention pooling).

Shapes (hardcoded): B=16, N=259 (L=256 + W-1=3), H=128, W=4, EPS=1e-6.
Reference:
    att[b,i,j] = 1 / (1 + sqrt(||x1[b,0,j,:] - x2[b,0,i,:]||^2 + EPS))
    x1_a[b,j] = sum_i att[b,i,j];  x2_a[b,i] = sum_j att[b,i,j]
    out_t[b,0,l,:] = sum_{k=0..3} x_t[b,0,l+k,:] * a_t[b,l+k],  l in [0,256)
Returns (out1, out2), each (16,1,256,128) fp32.

Strategy: data-parallel over batch, 2 batches per core across 8 cores.
Per batch: squared distances via Gram matmul on the PE
(s2 = sq1[j] + sq2[i] - 2*x2.x1, all three terms accumulated in PSUM by
three matmuls), sqrt on the scalar engine, fused (1/(1+x)) + row-sum as a
single custom DVE op, column sums via ones-matmul, and the W=4 sliding
window pooling as a banded-matrix matmul producing the output layout
directly.
"""

import numpy as np

import concourse.bass as bass
import concourse.tile as tile
from concourse import mybir
from concourse.bass_utils import run_bass_kernel_spmd

# --------------------------------------------------------------------------
# Custom DVE op: out = approx(1/(1 + x)) (seed + 1 Newton step, ~0.17% max
# rel err, zero-mean), accum_out = sum(out, free axis).
# --------------------------------------------------------------------------
import concourse.dve_ops as dve_ops
from concourse.dve_spec import Spec, Src0, C0, C1, One, AluOp, Bin, lower, _has_src1
from concourse.dve_ops import DveOp, OPS
from concourse.dve_uop import DveOpSpec

_S = Src0 + One
_nt = Bin(AluOp.BITWISE_NOT, _S, _S)
_y0 = _nt * C0
_BODY = _y0 * (C1 - _S * _y0)


def _recip_ref(in0, in1, s0, s1, imm2):
    S = (in0.astype(np.float32) + np.float32(1.0)).astype(np.float32)
    nt = (~S.view(np.int32)).view(np.float32)
    y0 = nt * np.float32(s0)
    return y0 * (np.float32(s1) - S * y0)


def _register_recip_op():
    name = "ADD1_RECIP_SUM_ANT"
    for existing in OPS:
        if existing.name == name:
            return existing
    spec = Spec(body=_BODY, accum=AluOp.ADD, reference=_recip_ref)
    op = DveOp(name, spec, subdim=False, uops_sha={})
    OPS.append(op)
    dve_ops._SUB_OPCODE_FOR_NAME[name] = dve_ops._CUSTOM_DVE_ROW_BASE + len(OPS) - 1
    for ver in ("v3", "v4"):
        op.uops_sha[ver] = DveOpSpec(
            name=name,
            opcode=dve_ops.get_dve_sub_opcode(name),
            uops=lower(spec, ver=ver),
            rd1_en=_has_src1(spec),
        ).sha(ver)
    return op


RECIP_OP = _register_recip_op()
RECIP_C0 = -0.23549792
RECIP_C1 = 2.0017324

# --------------------------------------------------------------------------
# Problem constants
# --------------------------------------------------------------------------
B, L, W, H = 16, 256, 4, 128
N = L + W - 1  # 259
EPS = 1e-6
NCORES = 8
BPC = B // NCORES  # batches per core = 2

f32 = mybir.dt.float32
f32r = mybir.dt.float32r
AF = mybir.ActivationFunctionType
ALU = mybir.AluOpType

# i-chunk decomposition of N=259: two full 128-partition chunks + 3 leftover
CHUNKS = [(0, 128), (128, 128), (256, 3)]
# f32r matmuls require an even moving (free) dim; pad 259 -> 260. The padded
# column holds garbage and is excluded everywhere it would matter.
NP = 260


def _host_consts():
    """DMA-able constant tables: identity, band main, band boundary."""
    ident = np.eye(128, dtype=np.float32)
    band = np.zeros((128, 128), dtype=np.float32)  # band[n, l] = 1 if l<=n<=l+3
    for n in range(128):
        lo = max(0, n - (W - 1))
        band[n, lo : n + 1] = 1.0
    bandb = np.zeros((128, 128), dtype=np.float32)
    for r in range(W - 1):  # boundary rows n = 128+r relative to l in [0,128)
        bandb[r, 125 + r : 128] = 1.0
    return np.stack([ident, band, bandb])  # (3, 128, 128)


def build_nc():
    nc = bass.Bass()
    x1_in = nc.dram_tensor("x1", [BPC, 1, N, H], f32, kind="ExternalInput")
    x2_in = nc.dram_tensor("x2", [BPC, 1, N, H], f32, kind="ExternalInput")
    consts_in = nc.dram_tensor("consts", [3, 128, 128], f32, kind="ExternalInput")
    out1_d = nc.dram_tensor("out1", [BPC, 1, L, H], f32, kind="ExternalOutput")
    out2_d = nc.dram_tensor("out2", [BPC, 1, L, H], f32, kind="ExternalOutput")

    xin = {0: x1_in, 1: x2_in}
    outd = {0: out1_d, 1: out2_d}

    with tile.TileContext(nc) as tc:
        with (
            tc.tile_pool(name="singles", bufs=1) as singles,
            tc.tile_pool(name="work", bufs=1) as work,
            tc.tile_pool(name="epool", bufs=2) as epool,
            tc.tile_pool(name="attpool", bufs=3) as attpool,
            tc.tile_pool(name="tp_ps", bufs=2, space="PSUM") as tp_ps,
            tc.tile_pool(name="gram_ps", bufs=3, space="PSUM") as gram_ps,
            tc.tile_pool(name="row_ps", bufs=1, space="PSUM") as row_ps,
            tc.tile_pool(name="acol_ps", bufs=1, space="PSUM") as acol_ps,
            tc.tile_pool(name="band_ps", bufs=1, space="PSUM") as band_ps,
        ):
            # ---- constants ----
            ident = singles.tile([128, 128], f32, tag="ident")
            band32 = singles.tile([128, 128], f32, tag="band32")
            bandb32 = singles.tile([3, 128], f32, tag="bandb32")
            band = singles.tile([128, 128], f32r, tag="band")
            bandb = singles.tile([3, 128], f32r, tag="bandb")
            ones = singles.tile([128, 512], f32r, tag="ones")
            ones32 = singles.tile([1, 1], f32, tag="ones32")
            epsb = singles.tile([128, 1], f32, tag="epsb")
            nc.sync.dma_start(out=ident, in_=consts_in[0])
            nc.sync.dma_start(out=band32, in_=consts_in[1])
            nc.sync.dma_start(out=bandb32, in_=consts_in[2, 0:3, :])
            nc.vector.tensor_copy(band[:, :], band32[:, :])
            nc.vector.tensor_copy(bandb[:, :], bandb32[:, :])
            ones_f = singles.tile([128, 512], f32, tag="ones_f")
            nc.gpsimd.memset(ones_f[:, :], 1.0)
            nc.vector.tensor_copy(ones[:, :], ones_f[:, :])
            nc.gpsimd.memset(ones32[:, :], 1.0)
            nc.gpsimd.memset(epsb[:, :], EPS)

            # ---- inputs, natural layout ----
            # x_nd[t]: (p, b, c, h) with n = c*128 + p   (c = 0,1)
            # x_l[t]:  (r, b, h)    with n = 256 + r
            x_nd, x_l = {}, {}
            for t in (0, 1):
                x_nd[t] = work.tile([128, BPC, 2, H], f32, tag=f"x{t}nd", name=f"x{t}nd")
                x_l[t] = work.tile([3, BPC, H], f32, tag=f"x{t}l", name=f"x{t}l")
                for b in range(BPC):
                    nc.sync.dma_start(
                        out=x_nd[t][:, b, :, :],
                        in_=xin[t][b, 0, 0:L, :].rearrange(
                            "(c p) h -> p c h", c=2
                        ),
                    )
                    nc.sync.dma_start(
                        out=x_l[t][:, b, :], in_=xin[t][b, 0, L:N, :]
                    )

            # ---- transposes to d-layout + squares, per (tensor, batch) ----
            # x_dn[t][b]: (128, 259)  d on partitions
            # xsq[t][b]:  (128, 259)  elementwise square of x_dn
            # x2 path is pre-scaled by -2 for the Gram term.
            x_dn = {0: {}, 1: {}}
            xsq = {0: {}, 1: {}}
            for t in (0, 1):
                for b in range(BPC):
                    tp = tp_ps.tile([128, N], f32, tag="tp")
                    for c in (0, 1):
                        nc.tensor.transpose(
                            tp[:, c * 128 : (c + 1) * 128],
                            x_nd[t][:, b, c, :],
                            ident[:, :],
                        )
                    nc.tensor.transpose(
                        tp[:, 256:259], x_l[t][:, b, :], ident[0:3, 0:3]
                    )
                    dn = work.tile([128, NP], f32r, tag=f"x{t}dn{b}", name=f"x{t}dn{b}")
                    if t == 0:
                        nc.scalar.copy(dn[:, 0:N], tp[:, :])
                        sq = work.tile([128, NP], f32r, tag=f"x{t}sq{b}", name=f"x{t}sq{b}")
                        nc.scalar.activation(sq[:, 0:N], tp[:, :], AF.Square)
                        xsq[t][b] = sq
                    else:
                        # -2 * x2 in d-layout (DVE, psum->sbuf)
                        nc.vector.tensor_scalar(
                            out=dn[:, 0:N], in0=tp[:, :],
                            scalar1=-2.0, scalar2=None, op0=ALU.mult,
                        )
                    x_dn[t][b] = dn

            # ---- attention chunks ----
            # psum = -2*x2.x1 + sq1[j]; e = sqrt(psum + (sq2[i]+EPS) bias)
            # att = 1/(1+e) (custom op) with accum -> x2_a columns
            # colsum ones-matmul -> x1_a row
            a_cols = {0: {}, 1: {}}  # a_cols[t][b]: (128, 4) sbuf, cols=chunks
            for t in (0, 1):
                for b in range(BPC):
                    a_cols[t][b] = work.tile([128, 4], f32, tag=f"a{t}c{b}", name=f"a{t}c{b}")

            # sq2e[b]: (128, 4) per-chunk columns of sum_d x2[n,d]^2 + EPS
            sq2e = {}
            sqscr = {}
            for b in range(BPC):
                sq2e[b] = work.tile([128, 4], f32, tag=f"sq2e{b}", name=f"sq2e{b}")
                sqscr[b] = work.tile([128, H], f32, tag="sqscr", name=f"sqscr{b}")
                for ci, (i0, P) in enumerate(CHUNKS):
                    if P == 128:
                        src = x_nd[1][:, b, i0 // 128, :]
                    else:
                        src = x_l[1][:, b, :]
                    nc.scalar.activation(
                        sqscr[b][0:P, :], src, AF.Square,
                        accum_out=sq2e[b][0:P, ci : ci + 1],
                    )
                nc.vector.tensor_scalar(
                    out=sq2e[b][:, :], in0=sq2e[b][:, :],
                    scalar1=EPS, scalar2=None, op0=ALU.add,
                )

            x1row_sb = {}
            for b in range(BPC):
                rowp = row_ps.tile([1, NP], f32, tag="x1row")
                for ci, (i0, P) in enumerate(CHUNKS):
                    g = gram_ps.tile([128, NP], f32, tag="gram")
                    # -2 * x2[:,i] . x1[:,j]
                    nc.tensor.matmul(
                        g[0:P, :],
                        x_dn[1][b][:, i0 : i0 + P],
                        x_dn[0][b][:, :],
                        start=True, stop=False,
                    )
                    # + sq1[j] broadcast over i  (ones^T @ xsq1)
                    nc.tensor.matmul(
                        g[0:P, :],
                        ones[:, 0:P],
                        xsq[0][b][:, :],
                        start=False, stop=True,
                    )
                    e = epool.tile([128, NP], f32, tag="e")
                    nc.scalar.activation(
                        e[0:P, :], g[0:P, :], AF.Sqrt,
                        bias=sq2e[b][0:P, ci : ci + 1],
                    )
                    att = attpool.tile([128, NP], f32r, tag="att")
                    nc.vector._custom_dve(
                        RECIP_OP,
                        out=att[0:P, 0:N], in0=e[0:P, 0:N],
                        s0=RECIP_C0, s1=RECIP_C1,
                        accum_out=a_cols[1][b][0:P, ci : ci + 1],
                    )

                    # x1_a row accumulation: ones_col^T @ att
                    nc.tensor.matmul(
                        rowp[:, :],
                        ones[0:P, 0:1],
                        att[0:P, :],
                        start=(ci == 0), stop=(ci == 2),
                    )
                row_sb = work.tile([1, NP], f32, tag="x1row_sb")
                nc.scalar.copy(row_sb[:, :], rowp[:, :])
                x1row_sb[b] = row_sb

            # x1_a row -> per-partition columns via tiny K=1 matmuls
            for b in range(BPC):
                ac = acol_ps.tile([128, 4], f32, tag="acolp")
                for ci, (i0, P) in enumerate(CHUNKS):
                    nc.tensor.matmul(
                        ac[0:P, ci : ci + 1],
                        x1row_sb[b][:, i0 : i0 + P],
                        ones32[0:1, 0:1],
                        start=True, stop=True,
                    )
                nc.vector.tensor_copy(a_cols[0][b][:, :], ac[:, :])

            # ---- weighted sliding-window pooling via banded matmul ----
            # wx = x * a (per-partition scalars); out[l, b, h] accumulated in
            # one psum bank per tensor, then one copy + one DMA out.
            for t in (0, 1):
                wx = work.tile([128, BPC, 2, H], f32r, tag=f"wx{t}", name=f"wx{t}")
                wxl = work.tile([3, BPC, H], f32r, tag=f"wxl{t}", name=f"wxl{t}")
                for b in range(BPC):
                    for c in (0, 1):
                        nc.vector.tensor_scalar(
                            out=wx[:, b, c, :], in0=x_nd[t][:, b, c, :],
                            scalar1=a_cols[t][b][:, c : c + 1],
                            scalar2=None, op0=ALU.mult,
                        )
                    nc.vector.tensor_scalar(
                        out=wxl[:, b, :], in0=x_l[t][:, b, :],
                        scalar1=a_cols[t][b][0:3, 2:3],
                        scalar2=None, op0=ALU.mult,
                    )
                bp = band_ps.tile([128, 2, BPC, H], f32, tag="bandp")
                # l-chunk 0: main rows n=0..127 (wx c=0), boundary n=128..130
                nc.tensor.matmul(
                    bp[:, 0, :, :],
                    band[:, :],
                    wx[:, :, 0, :],
                    start=True, stop=False,
                )
                nc.tensor.matmul(
                    bp[:, 0, :, :],
                    bandb[0:3, :],
                    wx[0:3, :, 1, :],
                    start=False, stop=True,
                )
                # l-chunk 1: main rows n=128..255 (wx c=1), boundary n=256..258
                nc.tensor.matmul(
                    bp[:, 1, :, :],
                    band[:, :],
                    wx[:, :, 1, :],
                    start=True, stop=False,
                )
                nc.tensor.matmul(
                    bp[:, 1, :, :],
                    bandb[0:3, :],
                    wxl[:, :, :],
                    start=False, stop=True,
                )
                osb = work.tile([128, 2, BPC, H], f32, tag=f"osb{t}")
                nc.vector.tensor_copy(osb[:, :, :, :], bp[:, :, :, :])
                for b in range(BPC):
                    nc.sync.dma_start(
                        out=outd[t][b, 0, :, :].rearrange(
                            "(lc p) h -> p lc h", lc=2
                        ),
                        in_=osb[:, :, b, :],
                    )
    # TRN2 allows at most 1 sem wait per instruction (2 on EventSemaphore);
    # Tile can attach more — split them like Bacc.compile does.
    import bass_rust
    from concourse import mybir as _mybir
    bass_rust.generate_event_semaphores(nc)
    _mybir.codegen_inst_isa_subclasses(nc)
    return nc


_NC_CACHE = {}


def _get_nc():
    if "nc" not in _NC_CACHE:
        _NC_CACHE["nc"] = build_nc()
    return _NC_CACHE["nc"]


def _run(x1, x2, **kwargs):
    x1 = np.ascontiguousarray(np.asarray(x1), dtype=np.float32)
    x2 = np.ascontiguousarray(np.asarray(x2), dtype=np.float32)
    consts = _host_consts()
    nc = _get_nc()
    core_ids = list(range(NCORES))
    in_maps = [
        {
            "x1": x1[c * BPC : (c + 1) * BPC],
            "x2": x2[c * BPC : (c + 1) * BPC],
            "consts": consts,
        }
        for c in core_ids
    ]
    br = run_bass_kernel_spmd(nc, in_maps, core_ids, **kwargs)
    out1 = np.concatenate([r["out1"] for r in br.results], axis=0)
    out2 = np.concatenate([r["out2"] for r in br.results], axis=0)
    return (out1, out2), br


def kernel(x1, x2):
    (out1, out2), _ = _run(x1, x2)
    return (out1, out2)


if __name__ == "__main__":
    rng = np.random.default_rng(0)
    x1 = rng.standard_normal((B, 1, N, H)).astype(np.float32)
    x2 = rng.standard_normal((B, 1, N, H)).astype(np.float32)
    o1, o2 = kernel(x1, x2)
    print("out shapes:", o1.shape, o2.shape)
